# revision 1
# baseline (speedup 1.0000x reference)
"""Class-balanced softmax cross-entropy loss on 8 Trainium2 NeuronCores.

Math (per the reference nn.Module):
  counts N_c   = histogram of target over classes (whole batch)
  weights w_c  = (1-beta)/(1-beta^N_c), 0 where N_c == 0
  logp         = log_softmax(logits, axis=C)
  loss         = -sum_pix w[t] * logp[t_pix] / sum_pix w[t]

Decomposition used here: per core (data-parallel over batch B=8, one batch
item per core) compute per-class partials
  N_c = sum(target == c)
  A_c = sum_{target==c} logits[c]
  B_c = sum_{target==c} lse          (lse = log(sum_c' exp(logits[c'])))
Then on host: N = sum_cores N_c; w from N;
  loss = -(w . (A - B)) / (w . N)
No collectives needed; each core returns 3*19 floats.
"""

import numpy as np
from contextlib import ExitStack
import sys

for _p in ("/opt/trn_rl_repo",):
    if _p not in sys.path:
        sys.path.insert(0, _p)

from concourse import bass, mybir, tile
from concourse.bass_utils import run_bass_kernel_spmd

NCLASS = 19
BETA = 0.999
NCORES = 8
HW = 512 * 1024          # pixels per batch item (= per core)
P = 128                  # SBUF partitions
COLS = HW // P           # 4096
F = 512                  # free-dim chunk
NCHUNK = COLS // F       # 8

f32 = mybir.dt.float32
i32 = mybir.dt.int32
AF = mybir.ActivationFunctionType
ALU = mybir.AluOpType

# accumulator column layout: [A (NCHUNK*NCLASS) | B (...) | N (...)]
SEC = NCHUNK * NCLASS          # 152
ACC_COLS = 3 * SEC             # 456


def _build():
    """Raw-bass pipeline with manual semaphores.

    Engine roles per chunk k (buffer half h=k%2):
      ACT: issue X/T DMAs, exp x19, log; DVE: reduce(sumexp), 57 stt/ts
      accumulations; PE: final partition-reduce matmul.
    Cross-engine edges via explicit wait_ge/then_inc; within-engine order is
    program order. Transitive implications (exp done => X landed) are used
    to keep the wait count low.
    """
    nc = bass.Bass()
    logits = nc.declare_dram_parameter("logits", [NCLASS, P, COLS], f32, isOutput=False)
    target = nc.declare_dram_parameter("target", [P, COLS], i32, isOutput=False)
    out = nc.declare_dram_parameter("out", [1, ACC_COLS], f32, isOutput=True)

    EF = NCLASS * F
    X2 = nc.alloc_sbuf_tensor("X2", [P, 2 * EF], f32)
    E2 = nc.alloc_sbuf_tensor("E2", [P, 2 * EF], f32)
    Ti2 = nc.alloc_sbuf_tensor("Ti2", [P, 2 * F], i32)
    Tf2 = nc.alloc_sbuf_tensor("Tf2", [P, 2 * F], f32)
    S2 = nc.alloc_sbuf_tensor("S2", [P, 2 * F], f32)
    L2 = nc.alloc_sbuf_tensor("L2", [P, 2 * F], f32)
    junk = nc.alloc_sbuf_tensor("junk", [P, F], f32)
    ABN = nc.alloc_sbuf_tensor("ABN", [P, ACC_COLS], f32)
    ones = nc.alloc_sbuf_tensor("ones", [P, 1], f32)
    ones_f = nc.alloc_sbuf_tensor("ones_f", [P, F], f32)
    res = nc.alloc_sbuf_tensor("res", [1, ACC_COLS], f32)
    ps = nc.alloc_psum_tensor("ps", [1, ACC_COLS], f32)

    with (
        nc.Block() as block,
        nc.semaphore("sem_x") as sem_x,
        nc.semaphore("sem_t") as sem_t,
        nc.semaphore("sem_exp") as sem_exp,
        nc.semaphore("sem_red") as sem_red,
        nc.semaphore("sem_log") as sem_log,
        nc.semaphore("sem_done") as sem_done,
        nc.semaphore("sem_mm") as sem_mm,
        nc.semaphore("sem_out") as sem_out,
    ):
        @block.scalar
        def _(act):
            for k in range(NCHUNK):
                h = k % 2
                if k >= 2:
                    act.wait_ge(sem_done, k - 1)   # bufs of chunk k-2 free
                act.dma_start(
                    X2[:, h * EF:(h + 1) * EF].rearrange("p (c f) -> p c f", c=NCLASS),
                    logits[:, :, k * F:(k + 1) * F].rearrange("c p f -> p c f"),
                ).then_inc(sem_x, 16)
                act.dma_start(
                    Ti2[:, h * F:(h + 1) * F], target[:, k * F:(k + 1) * F],
                ).then_inc(sem_t, 16)
                act.wait_ge(sem_x, 16 * (k + 1))
                for c in range(NCLASS):
                    ins = act.activation(
                        E2[:, h * EF + c * F: h * EF + (c + 1) * F],
                        X2[:, h * EF + c * F: h * EF + (c + 1) * F], AF.Exp)
                    if c == NCLASS - 1:
                        ins.then_inc(sem_exp, 1)
                act.wait_ge(sem_red, k + 1)
                act.activation(
                    L2[:, h * F:(h + 1) * F], S2[:, h * F:(h + 1) * F], AF.Ln,
                ).then_inc(sem_log, 1)
            # tail: psum -> sbuf -> dram
            act.wait_ge(sem_mm, 1)
            act.copy(res[:], ps[:])
            act.dma_start(out[:, :], res[:]).then_inc(sem_out, 16)
            act.wait_ge(sem_out, 16)

        @block.vector
        def _(dve):
            dve.memset(ABN[:], 0.0)
            dve.memset(ones[:], 1.0)
            dve.memset(ones_f[:], 1.0)
            for k in range(NCHUNK):
                h = k % 2
                dve.wait_ge(sem_exp, k + 1)   # E ready (implies X landed)
                dve.tensor_reduce(
                    S2[:, h * F:(h + 1) * F],
                    E2[:, h * EF:(h + 1) * EF].rearrange("p (c f) -> p f c", c=NCLASS),
                    axis=mybir.AxisListType.X, op=ALU.add,
                ).then_inc(sem_red, 1)
                dve.wait_ge(sem_t, 16 * (k + 1))
                Ti = Tf2[:, h * F:(h + 1) * F]
                dve.tensor_copy(Ti[:], Ti2[:, h * F:(h + 1) * F])
                for c in range(NCLASS):
                    dve.scalar_tensor_tensor(
                        out=junk[:], in0=Ti[:], scalar=float(c),
                        in1=X2[:, h * EF + c * F: h * EF + (c + 1) * F],
                        op0=ALU.is_equal, op1=ALU.mult,
                        accum_out=ABN[:, 0 * SEC + k * NCLASS + c: 0 * SEC + k * NCLASS + c + 1])
                dve.wait_ge(sem_log, k + 1)
                LSE = L2[:, h * F:(h + 1) * F]
                for c in range(NCLASS):
                    dve.scalar_tensor_tensor(
                        out=junk[:], in0=Ti[:], scalar=float(c), in1=LSE[:],
                        op0=ALU.is_equal, op1=ALU.mult,
                        accum_out=ABN[:, 1 * SEC + k * NCLASS + c: 1 * SEC + k * NCLASS + c + 1])
                for c in range(NCLASS):
                    # counts: single-src tensor_scalar runs in 2x_2P mode;
                    # op1 is the accum reduce op (add)
                    ins = dve.tensor_scalar(
                        out=junk[:], in0=Ti[:], scalar1=float(c), scalar2=None,
                        op0=ALU.is_equal, op1=ALU.add,
                        accum_out=ABN[:, 2 * SEC + k * NCLASS + c: 2 * SEC + k * NCLASS + c + 1])
                    if c == NCLASS - 1:
                        ins.then_inc(sem_done, 1)

        @block.tensor
        def _(pe):
            pe.wait_ge(sem_done, NCHUNK)
            pe.matmul(ps[:], lhsT=ones[:], rhs=ABN[:], start=True, stop=True).then_inc(sem_mm, 1)

    return nc


def _build_tile_unused():
    nc = bass.Bass()
    logits = nc.declare_dram_parameter("logits", [NCLASS, P, COLS], f32, isOutput=False)
    target = nc.declare_dram_parameter("target", [P, COLS], i32, isOutput=False)
    out = nc.declare_dram_parameter("out", [1, ACC_COLS], f32, isOutput=True)

    with ExitStack() as ctx:
        tc = ctx.enter_context(tile.TileContext(nc))
        xpool = ctx.enter_context(tc.tile_pool(name="x", bufs=2))
        tpool = ctx.enter_context(tc.tile_pool(name="t", bufs=2))
        accpool = ctx.enter_context(tc.tile_pool(name="acc", bufs=1))
        pspool = ctx.enter_context(tc.tile_pool(name="ps", bufs=1, space="PSUM"))

        EF = NCLASS * F
        ABN = accpool.tile([P, ACC_COLS], f32)
        nc.vector.memset(ABN[:], 0.0)
        ones = accpool.tile([P, 1], f32)
        nc.vector.memset(ones[:], 1.0)
        # persistent manually double-buffered scratch (avoids Tile pool
        # release-waits, which overflow the 1-sync-wait ISA limit)
        Ebuf = accpool.tile([P, 2 * EF], f32)
        Sbuf = accpool.tile([P, 2 * F], f32)
        Lbuf = accpool.tile([P, 2 * F], f32)
        junk = accpool.tile([P, F], f32)
        pabs = accpool.tile([P, 1], f32)   # DVE absorber dst
        pdve = accpool.tile([P, 1], f32)   # DVE->ACT probe src
        pscr = accpool.tile([P, 1], f32)   # ACT probe dst

        probes = {}
        for k in range(NCHUNK):
            h = k % 2
            X = xpool.tile([P, EF], f32, tag="x")
            xdma = nc.scalar.dma_start(
                X[:].rearrange("p (c f) -> p c f", c=NCLASS),
                logits[:, :, k * F:(k + 1) * F].rearrange("c p f -> p c f"))
            Ti = tpool.tile([P, F], i32, tag="ti")
            tdma = nc.scalar.dma_start(Ti[:], target[:, k * F:(k + 1) * F])
            if k >= 2:
                # Order this chunk's DMAs after the probe that made ACT
                # observe DVE's consumption of the recycled buffers, so the
                # DMACopy needs no extra sync-wait (1-wait ISA limit).
                tile.add_dep_helper(xdma.ins, probes[k - 2], reason="recycle absorb")
                tile.add_dep_helper(tdma.ins, probes[k - 2], reason="recycle absorb")

            E = Ebuf[:, h * EF:(h + 1) * EF]
            for c in range(NCLASS):
                nc.scalar.activation(E[:, c * F:(c + 1) * F], X[:, c * F:(c + 1) * F], AF.Exp)

            S = Sbuf[:, h * F:(h + 1) * F]
            nc.vector.tensor_reduce(
                S[:], E[:].rearrange("p (c f) -> p f c", c=NCLASS),
                axis=mybir.AxisListType.X, op=ALU.add)
            LSE = Lbuf[:, h * F:(h + 1) * F]
            log_ins = nc.scalar.activation(LSE[:], S[:], AF.Ln).ins

            # Drain instructions accept many sync-waits; use one as the
            # absorber for ALL of this chunk's cross-engine edges so every
            # following DVE instruction needs at most its self-wait.
            dr = nc.vector.drain()
            tile.add_dep_helper(dr.ins, xdma.ins, reason="absorb x dma")
            tile.add_dep_helper(dr.ins, tdma.ins, reason="absorb t dma")
            tile.add_dep_helper(dr.ins, log_ins, reason="absorb log")
            for c in range(NCLASS):
                # A_c partial: sum over free of (T==c)*logits_c
                stt = nc.vector.scalar_tensor_tensor(
                    out=junk[:], in0=Ti[:], scalar=float(c), in1=X[:, c * F:(c + 1) * F],
                    op0=ALU.is_equal, op1=ALU.mult,
                    accum_out=ABN[:, 0 * SEC + k * NCLASS + c: 0 * SEC + k * NCLASS + c + 1])
                if c == 0:
                    # force the drain ahead of the whole stt block (ordered
                    # among themselves by the junk WAW chain)
                    tile.add_dep_helper(stt.ins, dr.ins, reason="stt after drain")
            for c in range(NCLASS):
                # B_c partial: sum over free of (T==c)*lse
                nc.vector.scalar_tensor_tensor(
                    out=junk[:], in0=Ti[:], scalar=float(c), in1=LSE[:],
                    op0=ALU.is_equal, op1=ALU.mult,
                    accum_out=ABN[:, 1 * SEC + k * NCLASS + c: 1 * SEC + k * NCLASS + c + 1])
            for c in range(NCLASS):
                # N_c partial: sum over free of (T==c)
                nc.vector.tensor_scalar(
                    out=junk[:], in0=Ti[:], scalar1=float(c), scalar2=1.0,
                    op0=ALU.is_equal, op1=ALU.mult,
                    accum_out=ABN[:, 2 * SEC + k * NCLASS + c: 2 * SEC + k * NCLASS + c + 1])
            nc.vector.tensor_copy(pdve[:], junk[:, 0:1])
            probes[k] = nc.scalar.copy(pscr[:], pdve[:]).ins

        ps = pspool.tile([1, ACC_COLS], f32)
        mm = nc.tensor.matmul(ps[:], lhsT=ones[:], rhs=ABN[:], start=True, stop=True)
        dr2 = nc.scalar.drain()
        tile.add_dep_helper(dr2.ins, mm.ins, reason="absorb matmul")
        res = accpool.tile([1, ACC_COLS], f32)
        nc.scalar.copy(res[:], ps[:])
        nc.scalar.dma_start(out[:, :], res[:])

    return nc


_CACHE = {}


def _get_nc():
    if "nc" not in _CACHE:
        _CACHE["nc"] = _build()
    return _CACHE["nc"]


def _run(logits, target, trace=False):
    nc = _get_nc()
    in_maps = []
    for i in range(NCORES):
        in_maps.append({
            "logits": np.ascontiguousarray(logits[i].reshape(NCLASS, P, COLS)),
            "target": np.ascontiguousarray(target[i].reshape(P, COLS)),
        })
    r = run_bass_kernel_spmd(nc, in_maps, core_ids=list(range(NCORES)), trace=trace)
    return r


def _combine(results):
    A = np.zeros(NCLASS, np.float64)
    B = np.zeros(NCLASS, np.float64)
    N = np.zeros(NCLASS, np.float64)
    for i in range(NCORES):
        r = results[i]["out"].astype(np.float64).reshape(3, NCHUNK, NCLASS).sum(axis=1)
        A += r[0]
        B += r[1]
        N += r[2]
    w = np.where(N > 0, (1.0 - BETA) / (1.0 - BETA ** N), 0.0)
    num = float((w * (A - B)).sum())
    den = float((w * N).sum())
    return np.float32(-num / den)


def kernel(logits, target):
    assert logits.shape == (NCORES, NCLASS, 512, 1024) and logits.dtype == np.float32
    assert target.shape == (NCORES, 512, 1024) and target.dtype == np.int32
    r = _run(logits, target, trace=False)
    return _combine(r.results)



# revision 5
# speedup vs baseline: 1.4721x; 1.4721x over previous
"""Class-balanced softmax cross-entropy loss on 8 Trainium2 NeuronCores.

Math (per the reference nn.Module):
  counts N_c   = histogram of target over classes (whole batch)
  weights w_c  = (1-beta)/(1-beta^N_c), 0 where N_c == 0
  logp         = log_softmax(logits, axis=C)
  loss         = -sum_pix w[t] * logp[t_pix] / sum_pix w[t]

Device computes, per class c (data-parallel over batch, 1 item/core):
  A_c = sum_{target==c} logits[c]
  B_c = sum_{target==c} lse          (lse = log(sum_c' exp(logits[c'])))
via one-hot mask matmuls on the TensorEngine: for each group of W=4
pixel-columns f, a [128pix -> 4*20] stationary payload (bf16 logit cols
+ lse col, group-interleaved so the group is one contiguous 80-col
slice) against a [128pix, 4*19] moving one-hot (also group-interleaved,
76 contiguous cols) accumulates block-diagonal per-class sums in PSUM
across all pixels.  N_c and the weights come from a host-side bincount
of target (exact); the host combines A, B, N into the scalar loss.

Group-interleaved layouts keep every DVE write inner-step-1 so the
one-hot runs in 4x mode and the casts in 2x mode.

Engine split per chunk (F=512 pixel-cols, 8 chunks/core):
  SP    : all input DMAs
  ACT   : exp (one big f32->bf16 instr), log -> lse (bf16)
  DVE   : int->bf16 target cast, 19x one-hot (bf16 4x mode),
          bf16 pairwise-tree sum of exp for sumexp (2x mode), part of
          the f32->bf16 logit cast
  GPSIMD: bulk of the f32->bf16 logit cast
  PE    : 128 group matmuls, PSUM-accumulated across the whole core
"""

import numpy as np
import sys

for _p in ("/opt/trn_rl_repo",):
    if _p not in sys.path:
        sys.path.insert(0, _p)

from concourse import bass, mybir
from concourse.bass_utils import run_bass_kernel_spmd

NCLASS = 19
BETA = 0.999
NCORES = 8
P = 128
COLS = 4096              # 512*1024 / 128
F = 512                  # pixel-cols per chunk
NCHUNK = COLS // F       # 8
W = 4                    # f-columns per matmul group
NGRP = F // W            # 128 groups per chunk, no tail
MW = W * NCLASS          # 76 moving (one-hot) cols per group
SW = W * (NCLASS + 1)    # 80 stationary (payload) cols per group
SPLIT_G = 12             # classes cast f32->bf16 on GPSIMD; rest on DVE

f32 = mybir.dt.float32
bf16 = mybir.dt.bfloat16
i32 = mybir.dt.int32
AF = mybir.ActivationFunctionType
ALU = mybir.AluOpType

EF = NCLASS * F          # 9728 elems/partition per chunk of X / E / M
RBW = (NCLASS + 1) * F   # 10240: payload elems/partition per chunk


def _build():
    nc = bass.Bass()
    logits = nc.declare_dram_parameter("logits", [NCLASS, P, COLS], f32, isOutput=False)
    target = nc.declare_dram_parameter("target", [P, COLS], i32, isOutput=False)
    out = nc.declare_dram_parameter("out", [SW, MW], f32, isOutput=True)

    X2 = nc.alloc_sbuf_tensor("X2", [P, 2 * EF], f32)      # raw logits chunks
    E2 = nc.alloc_sbuf_tensor("E2", [P, 2 * EF], bf16)     # exp(logits), tree scratch
    RB2 = nc.alloc_sbuf_tensor("RB2", [P, 2 * RBW], bf16)  # payload, grp-interleaved
    M2 = nc.alloc_sbuf_tensor("M2", [P, 2 * EF], bf16)     # one-hot, grp-interleaved
    T2 = nc.alloc_sbuf_tensor("T2", [P, 2 * F], i32)
    TB2 = nc.alloc_sbuf_tensor("TB2", [P, 2 * F], bf16)
    SCR = nc.alloc_sbuf_tensor("SCR", [P, F], bf16)        # tree level-1 scratch
    RES = nc.alloc_sbuf_tensor("RES", [SW, MW], f32)
    psm = nc.alloc_psum_tensor("psm", [SW, MW], f32)

    with (
        nc.Block() as block,
        nc.semaphore("sem_x") as sem_x,
        nc.semaphore("sem_t") as sem_t,
        nc.semaphore("sem_exp") as sem_exp,
        nc.semaphore("sem_tree") as sem_tree,
        nc.semaphore("sem_cast") as sem_cast,
        nc.semaphore("sem_m") as sem_m,
        nc.semaphore("sem_lse") as sem_lse,
        nc.semaphore("sem_pe") as sem_pe,
        nc.semaphore("sem_out") as sem_out,
    ):
        @block.sync
        def _(sp):
            for k in range(NCHUNK):
                h = k % 2
                if k >= 2:
                    # X2[h] free once exp(k-2) and both cast halves (k-2) done
                    sp.wait_ge(sem_exp, k - 1)
                    sp.wait_ge(sem_cast, 2 * (k - 1))
                sp.dma_start(
                    X2[:, h * EF:(h + 1) * EF].rearrange("p (c f) -> p c f", c=NCLASS),
                    logits[:, :, k * F:(k + 1) * F].rearrange("c p f -> p c f"),
                ).then_inc(sem_x, 16)
                if k >= 2:
                    # T2[h] free once DVE passed one-hot(k-2) (t-cast precedes it)
                    sp.wait_ge(sem_m, k - 1)
                sp.dma_start(
                    T2[:, h * F:(h + 1) * F], target[:, k * F:(k + 1) * F],
                ).then_inc(sem_t, 16)

        @block.scalar
        def _(act):
            for k in range(NCHUNK):
                h = k % 2
                act.wait_ge(sem_x, 16 * (k + 1))
                act.activation(
                    E2[:, h * EF:(h + 1) * EF], X2[:, h * EF:(h + 1) * EF], AF.Exp,
                ).then_inc(sem_exp, 1)
                if k >= 2:
                    act.wait_ge(sem_pe, k - 1)   # RB2[h] free (PE k-2 done)
                act.wait_ge(sem_tree, k + 1)
                # sumexp landed in E2[h] block 0 (bf16); lse -> payload q=19 slot
                RBh = RB2[:, h * RBW:(h + 1) * RBW].rearrange(
                    "p (g q w) -> p g q w", g=NGRP, q=NCLASS + 1)
                act.activation(
                    RBh[:, :, NCLASS, :],
                    E2[:, h * EF: h * EF + F].rearrange("p (g w) -> p g w", g=NGRP),
                    AF.Ln,
                ).then_inc(sem_lse, 1)
            # tail: psum -> sbuf -> dram
            act.wait_ge(sem_pe, NCHUNK)
            act.copy(RES[:], psm[:])
            act.dma_start(out[:, :], RES[:]).then_inc(sem_out, 16)
            act.wait_ge(sem_out, 16)

        @block.gpsimd
        def _(gp):
            for k in range(NCHUNK):
                h = k % 2
                gp.wait_ge(sem_x, 16 * (k + 1))
                if k >= 2:
                    gp.wait_ge(sem_pe, k - 1)    # RB2[h] free
                RBh = RB2[:, h * RBW:(h + 1) * RBW].rearrange(
                    "p (g q w) -> p g q w", g=NGRP, q=NCLASS + 1)
                Xh = X2[:, h * EF:(h + 1) * EF].rearrange(
                    "p (c g w) -> p c g w", c=NCLASS, g=NGRP)
                gp.tensor_copy(
                    RBh[:, :, 0:SPLIT_G, :].rearrange("p g q w -> p q g w"),
                    Xh[:, 0:SPLIT_G, :, :],
                ).then_inc(sem_cast, 1)

        @block.vector
        def _(dve):
            dve.memset(RES[:], 0.0)
            for k in range(NCHUNK):
                h = k % 2
                dve.wait_ge(sem_t, 16 * (k + 1))
                dve.tensor_copy(TB2[:, h * F:(h + 1) * F], T2[:, h * F:(h + 1) * F])
                dve.wait_ge(sem_x, 16 * (k + 1))
                if k >= 2:
                    dve.wait_ge(sem_pe, k - 1)   # RB2[h] and M2[h] free
                RBh = RB2[:, h * RBW:(h + 1) * RBW].rearrange(
                    "p (g q w) -> p g q w", g=NGRP, q=NCLASS + 1)
                Xh = X2[:, h * EF:(h + 1) * EF].rearrange(
                    "p (c g w) -> p c g w", c=NCLASS, g=NGRP)
                dve.tensor_copy(
                    RBh[:, :, SPLIT_G:NCLASS, :].rearrange("p g q w -> p q g w"),
                    Xh[:, SPLIT_G:NCLASS, :, :],
                ).then_inc(sem_cast, 1)
                Mh = M2[:, h * EF:(h + 1) * EF].rearrange(
                    "p (g c w) -> p g c w", g=NGRP, c=NCLASS)
                TBh = TB2[:, h * F:(h + 1) * F].rearrange(
                    "p (g w) -> p g w", g=NGRP)
                for c in range(NCLASS):
                    ins = dve.tensor_scalar(
                        out=Mh[:, :, c, :], in0=TBh[:],
                        scalar1=float(c), scalar2=None, op0=ALU.is_equal,
                        op1=ALU.bypass)
                    if c == NCLASS - 1:
                        ins.then_inc(sem_m, 1)
                # bf16 pairwise tree-sum of the 19 exp blocks -> block 0
                dve.wait_ge(sem_exp, k + 1)
                E = lambda b: E2[:, h * EF + b * F: h * EF + (b + 1) * F]
                def add(dst, a, b):
                    return dve.tensor_tensor(out=dst, in0=a, in1=b, op=ALU.add)
                add(SCR[:], E(0), E(1))                      # L1
                for j in range(1, 9):
                    add(E(2 * (j - 1)), E(2 * j), E(2 * j + 1))
                # partials: SCR,0,2,4,6,8,10,12,14 and 18
                add(E(1), SCR[:], E(0))                      # L2
                add(E(3), E(2), E(4))
                add(E(5), E(6), E(8))
                add(E(7), E(10), E(12))
                add(E(9), E(14), E(18))
                add(E(0), E(1), E(3))                        # L3
                add(E(2), E(5), E(7))
                add(E(1), E(0), E(2))                        # L4
                ins = add(E(0), E(1), E(9))                  # L5 -> block 0
                ins.then_inc(sem_tree, 1)

        @block.tensor
        def _(pe):
            for k in range(NCHUNK):
                h = k % 2
                pe.wait_ge(sem_m, k + 1)
                pe.wait_ge(sem_cast, 2 * (k + 1))
                pe.wait_ge(sem_lse, k + 1)
                for g in range(NGRP):
                    ins = pe.matmul(
                        psm[:],
                        lhsT=RB2[:, h * RBW + g * SW: h * RBW + (g + 1) * SW],
                        rhs=M2[:, h * EF + g * MW: h * EF + (g + 1) * MW],
                        start=(k == 0 and g == 0),
                        stop=(k == NCHUNK - 1 and g == NGRP - 1),
                    )
                    if g == NGRP - 1:
                        ins.then_inc(sem_pe, 1)

    return nc


_CACHE = {}


def _get_nc():
    if "nc" not in _CACHE:
        _CACHE["nc"] = _build()
    return _CACHE["nc"]


def _run(logits, target, trace=False):
    nc = _get_nc()
    in_maps = []
    for i in range(NCORES):
        in_maps.append({
            "logits": np.ascontiguousarray(logits[i].reshape(NCLASS, P, COLS)),
            "target": np.ascontiguousarray(target[i].reshape(P, COLS)),
        })
    r = run_bass_kernel_spmd(nc, in_maps, core_ids=list(range(NCORES)), trace=trace)
    return r


def _combine(results, target):
    # psum out layout: rows i = q*W + floc (payload), cols j = c*W + floc'
    # (one-hot); valid entries are the floc == floc' diagonals.
    A = np.zeros(NCLASS, np.float64)
    B = np.zeros(NCLASS, np.float64)
    c19 = np.arange(NCLASS)
    for i in range(NCORES):
        res = results[i]["out"].astype(np.float64)
        for floc in range(W):
            A += res[c19 * W + floc, c19 * W + floc]
            B += res[NCLASS * W + floc, c19 * W + floc]
    N = np.bincount(target.reshape(-1), minlength=NCLASS).astype(np.float64)
    w = np.where(N > 0, (1.0 - BETA) / (1.0 - BETA ** N), 0.0)
    num = float((w * (A - B)).sum())
    den = float((w * N).sum())
    return np.float32(-num / den)


def kernel(logits, target):
    assert logits.shape == (NCORES, NCLASS, 512, 1024) and logits.dtype == np.float32
    assert target.shape == (NCORES, 512, 1024) and target.dtype == np.int32
    r = _run(logits, target, trace=False)
    return _combine(r.results, target)


# revision 9
# speedup vs baseline: 2.8673x; 1.9477x over previous
"""Class-balanced softmax cross-entropy loss on 8 Trainium2 NeuronCores.

Math (per the reference nn.Module):
  counts N_c   = histogram of target over classes (whole batch)
  weights w_c  = (1-beta)/(1-beta^N_c), 0 where N_c == 0
  logp         = log_softmax(logits, axis=C)
  loss         = -sum_pix w[t] * logp[t_pix] / sum_pix w[t]

Device computes, per class c (data-parallel over batch, 1 item/core):
  A_c = sum_{target==c} logits[c]
  B_c = sum_{target==c} lse          (lse = log(sum_c' exp(logits[c'])))
via one-hot mask matmuls on the TensorEngine: for each group of W=4
pixel-columns f, a [128pix -> 4*20] stationary payload (bf16 logit cols
+ lse col, group-interleaved so the group is one contiguous 80-col
slice) against a [128pix, 4*19] moving one-hot (also group-interleaved,
76 contiguous cols) accumulates block-diagonal per-class sums in PSUM
across all pixels.  N_c and the weights come from a host-side bincount
of target (exact); the host combines A, B, N into the scalar loss.

Group-interleaved layouts keep every DVE write inner-step-1 so the
one-hot runs in 4x mode and the casts in 2x mode.

Engine split per chunk (F=512 pixel-cols, 8 chunks/core):
  SP    : all input DMAs
  ACT   : exp (one big f32->bf16 instr), log -> lse (bf16)
  DVE   : int->bf16 target cast, 19x one-hot (bf16 4x mode),
          bf16 pairwise-tree sum of exp for sumexp (2x mode), part of
          the f32->bf16 logit cast
  GPSIMD: bulk of the f32->bf16 logit cast
  PE    : 128 group matmuls, PSUM-accumulated across the whole core
"""

import numpy as np
import sys

for _p in ("/opt/trn_rl_repo",):
    if _p not in sys.path:
        sys.path.insert(0, _p)

from concourse import bass, mybir
from concourse.bass_utils import run_bass_kernel_spmd

NCLASS = 19
BETA = 0.999
NCORES = 8
P = 128
COLS = 4096              # 512*1024 / 128
F = 512                  # pixel-cols per chunk
NCHUNK = COLS // F       # 8
W = 4                    # f-columns per matmul group
NGRP = F // W            # 128 groups per chunk, no tail
MW = W * NCLASS          # 76 moving (one-hot) cols per group
SW = W * (NCLASS + 1)    # 80 stationary (payload) cols per group
SPLIT_D = 12             # classes cast f32->bf16 on DVE; rest on ACT
# (GPSIMD is deliberately unused: it shares an SBUF port with the Vector
# engine, and any sustained GPSIMD op halves DVE throughput.)

f32 = mybir.dt.float32
bf16 = mybir.dt.bfloat16
i32 = mybir.dt.int32
AF = mybir.ActivationFunctionType
ALU = mybir.AluOpType

EF = NCLASS * F          # 9728 elems/partition per chunk of X / E / M
RBW = (NCLASS + 1) * F   # 10240: payload elems/partition per chunk


def _build():
    nc = bass.Bass()
    logits = nc.declare_dram_parameter("logits", [NCLASS, P, COLS], f32, isOutput=False)
    target = nc.declare_dram_parameter("target", [P, COLS], i32, isOutput=False)
    out = nc.declare_dram_parameter("out", [SW, MW], f32, isOutput=True)

    X2 = nc.alloc_sbuf_tensor("X2", [P, 2 * EF], f32)      # raw logits chunks
    E2 = nc.alloc_sbuf_tensor("E2", [P, 2 * EF], bf16)     # exp(logits), tree scratch
    RB2 = nc.alloc_sbuf_tensor("RB2", [P, 2 * RBW], bf16)  # payload, grp-interleaved
    M2 = nc.alloc_sbuf_tensor("M2", [P, 2 * EF], bf16)     # one-hot, grp-interleaved
    T2 = nc.alloc_sbuf_tensor("T2", [P, 2 * F], i32)
    TB2 = nc.alloc_sbuf_tensor("TB2", [P, 2 * F], bf16)
    SCR = nc.alloc_sbuf_tensor("SCR", [P, F], bf16)        # tree level-1 scratch
    RES = nc.alloc_sbuf_tensor("RES", [SW, MW], f32)
    psm = nc.alloc_psum_tensor("psm", [SW, MW], f32)

    with (
        nc.Block() as block,
        nc.semaphore("sem_x") as sem_x,
        nc.semaphore("sem_t") as sem_t,
        nc.semaphore("sem_exp") as sem_exp,
        nc.semaphore("sem_tree") as sem_tree,
        nc.semaphore("sem_cast") as sem_cast,
        nc.semaphore("sem_m") as sem_m,
        nc.semaphore("sem_lse") as sem_lse,
        nc.semaphore("sem_pe") as sem_pe,
        nc.semaphore("sem_out") as sem_out,
    ):
        @block.sync
        def _(sp):
            for k in range(NCHUNK):
                h = k % 2
                if k >= 2:
                    # X2[h] free once exp(k-2) and both cast halves (k-2) done
                    sp.wait_ge(sem_exp, k - 1)
                    sp.wait_ge(sem_cast, 2 * (k - 1))
                sp.dma_start(
                    X2[:, h * EF:(h + 1) * EF].rearrange("p (c f) -> p c f", c=NCLASS),
                    logits[:, :, k * F:(k + 1) * F].rearrange("c p f -> p c f"),
                ).then_inc(sem_x, 16)
                if k >= 2:
                    # T2[h] free once DVE passed one-hot(k-2) (t-cast precedes it)
                    sp.wait_ge(sem_m, k - 1)
                sp.dma_start(
                    T2[:, h * F:(h + 1) * F], target[:, k * F:(k + 1) * F],
                ).then_inc(sem_t, 16)

        @block.scalar
        def _(act):
            for k in range(NCHUNK):
                h = k % 2
                act.wait_ge(sem_x, 16 * (k + 1))
                act.activation(
                    E2[:, h * EF:(h + 1) * EF], X2[:, h * EF:(h + 1) * EF], AF.Exp,
                ).then_inc(sem_exp, 1)
                if k >= 2:
                    act.wait_ge(sem_pe, k - 1)   # RB2[h] free (PE k-2 done)
                RBh = RB2[:, h * RBW:(h + 1) * RBW].rearrange(
                    "p (g q w) -> p g q w", g=NGRP, q=NCLASS + 1)
                Xh = X2[:, h * EF:(h + 1) * EF].rearrange(
                    "p (c g w) -> p c g w", c=NCLASS, g=NGRP)
                act.copy(
                    RBh[:, :, SPLIT_D:NCLASS, :].rearrange("p g q w -> p q g w"),
                    Xh[:, SPLIT_D:NCLASS, :, :],
                ).then_inc(sem_cast, 1)
                act.wait_ge(sem_tree, k + 1)
                # sumexp landed in E2[h] block 0 (bf16); lse -> payload q=19 slot
                act.activation(
                    RBh[:, :, NCLASS, :],
                    E2[:, h * EF: h * EF + F].rearrange("p (g w) -> p g w", g=NGRP),
                    AF.Ln,
                ).then_inc(sem_lse, 1)
            # tail: psum -> sbuf -> dram
            act.wait_ge(sem_pe, NCHUNK)
            act.copy(RES[:], psm[:])
            act.dma_start(out[:, :], RES[:]).then_inc(sem_out, 16)
            act.wait_ge(sem_out, 16)

        @block.vector
        def _(dve):
            dve.memset(RES[:], 0.0)
            for k in range(NCHUNK):
                h = k % 2
                dve.wait_ge(sem_t, 16 * (k + 1))
                dve.tensor_copy(TB2[:, h * F:(h + 1) * F], T2[:, h * F:(h + 1) * F])
                dve.wait_ge(sem_x, 16 * (k + 1))
                if k >= 2:
                    dve.wait_ge(sem_pe, k - 1)   # RB2[h] and M2[h] free
                RBh = RB2[:, h * RBW:(h + 1) * RBW].rearrange(
                    "p (g q w) -> p g q w", g=NGRP, q=NCLASS + 1)
                Xh = X2[:, h * EF:(h + 1) * EF].rearrange(
                    "p (c g w) -> p c g w", c=NCLASS, g=NGRP)
                dve.tensor_copy(
                    RBh[:, :, 0:SPLIT_D, :].rearrange("p g q w -> p q g w"),
                    Xh[:, 0:SPLIT_D, :, :],
                ).then_inc(sem_cast, 1)
                Mh = M2[:, h * EF:(h + 1) * EF].rearrange(
                    "p (g c w) -> p g c w", g=NGRP, c=NCLASS)
                TBh = TB2[:, h * F:(h + 1) * F].rearrange(
                    "p (g w) -> p g w", g=NGRP)
                for c in range(NCLASS):
                    ins = dve.tensor_scalar(
                        out=Mh[:, :, c, :], in0=TBh[:],
                        scalar1=float(c), scalar2=None, op0=ALU.is_equal,
                        op1=ALU.bypass)
                    if c == NCLASS - 1:
                        ins.then_inc(sem_m, 1)
                # bf16 pairwise tree-sum of the 19 exp blocks -> block 0
                dve.wait_ge(sem_exp, k + 1)
                E = lambda b: E2[:, h * EF + b * F: h * EF + (b + 1) * F]
                def add(dst, a, b):
                    return dve.tensor_tensor(out=dst, in0=a, in1=b, op=ALU.add)
                add(SCR[:], E(0), E(1))                      # L1
                for j in range(1, 9):
                    add(E(2 * (j - 1)), E(2 * j), E(2 * j + 1))
                # partials: SCR,0,2,4,6,8,10,12,14 and 18
                add(E(1), SCR[:], E(0))                      # L2
                add(E(3), E(2), E(4))
                add(E(5), E(6), E(8))
                add(E(7), E(10), E(12))
                add(E(9), E(14), E(18))
                add(E(0), E(1), E(3))                        # L3
                add(E(2), E(5), E(7))
                add(E(1), E(0), E(2))                        # L4
                ins = add(E(0), E(1), E(9))                  # L5 -> block 0
                ins.then_inc(sem_tree, 1)

        @block.tensor
        def _(pe):
            for k in range(NCHUNK):
                h = k % 2
                pe.wait_ge(sem_m, k + 1)
                pe.wait_ge(sem_cast, 2 * (k + 1))
                pe.wait_ge(sem_lse, k + 1)
                for g in range(NGRP):
                    ins = pe.matmul(
                        psm[:],
                        lhsT=RB2[:, h * RBW + g * SW: h * RBW + (g + 1) * SW],
                        rhs=M2[:, h * EF + g * MW: h * EF + (g + 1) * MW],
                        start=(k == 0 and g == 0),
                        stop=(k == NCHUNK - 1 and g == NGRP - 1),
                    )
                    if g == NGRP - 1:
                        ins.then_inc(sem_pe, 1)

    return nc


_CACHE = {}


def _get_nc():
    if "nc" not in _CACHE:
        _CACHE["nc"] = _build()
    return _CACHE["nc"]


def _run(logits, target, trace=False):
    nc = _get_nc()
    in_maps = []
    for i in range(NCORES):
        in_maps.append({
            "logits": np.ascontiguousarray(logits[i].reshape(NCLASS, P, COLS)),
            "target": np.ascontiguousarray(target[i].reshape(P, COLS)),
        })
    r = run_bass_kernel_spmd(nc, in_maps, core_ids=list(range(NCORES)), trace=trace)
    return r


def _combine(results, target):
    # psum out layout: rows i = q*W + floc (payload), cols j = c*W + floc'
    # (one-hot); valid entries are the floc == floc' diagonals.
    A = np.zeros(NCLASS, np.float64)
    B = np.zeros(NCLASS, np.float64)
    c19 = np.arange(NCLASS)
    for i in range(NCORES):
        res = results[i]["out"].astype(np.float64)
        for floc in range(W):
            A += res[c19 * W + floc, c19 * W + floc]
            B += res[NCLASS * W + floc, c19 * W + floc]
    N = np.bincount(target.reshape(-1), minlength=NCLASS).astype(np.float64)
    w = np.where(N > 0, (1.0 - BETA) / (1.0 - BETA ** N), 0.0)
    num = float((w * (A - B)).sum())
    den = float((w * N).sum())
    return np.float32(-num / den)


def kernel(logits, target):
    assert logits.shape == (NCORES, NCLASS, 512, 1024) and logits.dtype == np.float32
    assert target.shape == (NCORES, 512, 1024) and target.dtype == np.int32
    r = _run(logits, target, trace=False)
    return _combine(r.results, target)


# revision 10
# speedup vs baseline: 3.0153x; 1.0516x over previous
"""Class-balanced softmax cross-entropy loss on 8 Trainium2 NeuronCores.

Math (per the reference nn.Module):
  counts N_c   = histogram of target over classes (whole batch)
  weights w_c  = (1-beta)/(1-beta^N_c), 0 where N_c == 0
  logp         = log_softmax(logits, axis=C)
  loss         = -sum_pix w[t] * logp[t_pix] / sum_pix w[t]

Device computes, per class c (data-parallel over batch, 1 item/core):
  A_c = sum_{target==c} logits[c]
  B_c = sum_{target==c} lse          (lse = log(sum_c' exp(logits[c'])))
via one-hot mask matmuls on the TensorEngine: for each group of W=4
pixel-columns f, a [128pix -> 4*20] stationary payload (bf16 logit cols
+ lse col, group-interleaved so the group is one contiguous 80-col
slice) against a [128pix, 4*19] moving one-hot (also group-interleaved,
76 contiguous cols) accumulates block-diagonal per-class sums in PSUM
across all pixels.  N_c and the weights come from a host-side bincount
of target (exact); the host combines A, B, N into the scalar loss.

Group-interleaved layouts keep every DVE write inner-step-1 so the
one-hot runs in 4x mode and the casts in 2x mode.

Engine split per chunk (variable-size chunks; small chunks at the start
and end shrink pipeline fill/drain):
  SP    : all input DMAs
  ACT   : exp (one big f32->bf16 instr), 7/19 of the f32->bf16 logit
          cast, log -> lse (bf16)
  DVE   : int->bf16 target cast, 19x one-hot (bf16 4x mode), 12/19 of
          the logit cast (2x mode), bf16 batched pairwise-tree sum of
          exp for sumexp (2x mode)
  PE    : F/4 group matmuls per chunk, PSUM-accumulated
  GPSIMD: deliberately unused — it shares an SBUF port with the Vector
          engine, and any sustained GPSIMD op halves DVE throughput.
"""

import numpy as np
import sys

for _p in ("/opt/trn_rl_repo",):
    if _p not in sys.path:
        sys.path.insert(0, _p)

from concourse import bass, mybir
from concourse.bass_utils import run_bass_kernel_spmd

NCLASS = 19
BETA = 0.999
NCORES = 8
P = 128
COLS = 4096              # 512*1024 / 128
FMAX = 512               # buffer slot size (largest chunk)
CHUNKS = [128, 256, 512, 512, 512, 512, 512, 512, 384, 256]
assert sum(CHUNKS) == COLS and all(f % 4 == 0 and f <= FMAX for f in CHUNKS)
NCHUNK = len(CHUNKS)
OFFS = [sum(CHUNKS[:i]) for i in range(NCHUNK)]
W = 4                    # f-columns per matmul group
MW = W * NCLASS          # 76 moving (one-hot) cols per group
SW = W * (NCLASS + 1)    # 80 stationary (payload) cols per group
SPLIT_D = 12             # classes cast f32->bf16 on DVE; rest on ACT

f32 = mybir.dt.float32
bf16 = mybir.dt.bfloat16
i32 = mybir.dt.int32
AF = mybir.ActivationFunctionType
ALU = mybir.AluOpType

EFM = NCLASS * FMAX      # X/E/M buffer slot elems per partition
RBM = (NCLASS + 1) * FMAX


def _build():
    nc = bass.Bass()
    logits = nc.declare_dram_parameter("logits", [NCLASS, P, COLS], f32, isOutput=False)
    target = nc.declare_dram_parameter("target", [P, COLS], i32, isOutput=False)
    out = nc.declare_dram_parameter("out", [SW, MW], f32, isOutput=True)

    X2 = nc.alloc_sbuf_tensor("X2", [P, 2 * EFM], f32)      # raw logits chunks
    E2 = nc.alloc_sbuf_tensor("E2", [P, 2 * EFM], bf16)     # exp(logits) + tree
    RB2 = nc.alloc_sbuf_tensor("RB2", [P, 2 * RBM], bf16)   # payload, grp-interleaved
    M2 = nc.alloc_sbuf_tensor("M2", [P, 2 * EFM], bf16)     # one-hot, grp-interleaved
    T2 = nc.alloc_sbuf_tensor("T2", [P, 2 * FMAX], i32)
    TB2 = nc.alloc_sbuf_tensor("TB2", [P, 2 * FMAX], bf16)
    SCR = nc.alloc_sbuf_tensor("SCR", [P, FMAX], bf16)      # tree L1 scratch
    RES = nc.alloc_sbuf_tensor("RES", [SW, MW], f32)
    psm = nc.alloc_psum_tensor("psm", [SW, MW], f32)

    with (
        nc.Block() as block,
        nc.semaphore("sem_x") as sem_x,
        nc.semaphore("sem_t") as sem_t,
        nc.semaphore("sem_exp") as sem_exp,
        nc.semaphore("sem_tree") as sem_tree,
        nc.semaphore("sem_cast") as sem_cast,
        nc.semaphore("sem_m") as sem_m,
        nc.semaphore("sem_lse") as sem_lse,
        nc.semaphore("sem_pe") as sem_pe,
        nc.semaphore("sem_out") as sem_out,
    ):
        # per-chunk views, parametrized by chunk index
        def views(k):
            h = k % 2
            F = CHUNKS[k]
            ng = F // W
            Xh = X2[:, h * EFM: h * EFM + NCLASS * F]
            Eh = E2[:, h * EFM: h * EFM + NCLASS * F]
            RBh = RB2[:, h * RBM: h * RBM + (NCLASS + 1) * F]
            Mh = M2[:, h * EFM: h * EFM + NCLASS * F]
            Th = T2[:, h * FMAX: h * FMAX + F]
            TBh = TB2[:, h * FMAX: h * FMAX + F]
            return h, F, ng, Xh, Eh, RBh, Mh, Th, TBh

        @block.sync
        def _(sp):
            for k in range(NCHUNK):
                _, F, ng, Xh, _, _, _, Th, _ = views(k)
                if k >= 2:
                    # X2[h] free once exp(k-2) and both cast halves (k-2) done
                    sp.wait_ge(sem_exp, k - 1)
                    sp.wait_ge(sem_cast, 2 * (k - 1))
                sp.dma_start(
                    Xh.rearrange("p (c f) -> p c f", c=NCLASS),
                    logits[:, :, OFFS[k]:OFFS[k] + F].rearrange("c p f -> p c f"),
                ).then_inc(sem_x, 16)
                if k >= 2:
                    # T2[h] free once DVE passed one-hot(k-2) (t-cast precedes it)
                    sp.wait_ge(sem_m, k - 1)
                sp.dma_start(
                    Th, target[:, OFFS[k]:OFFS[k] + F],
                ).then_inc(sem_t, 16)

        @block.scalar
        def _(act):
            for k in range(NCHUNK):
                _, F, ng, Xh, Eh, RBh, _, _, _ = views(k)
                act.wait_ge(sem_x, 16 * (k + 1))
                act.activation(Eh, Xh, AF.Exp).then_inc(sem_exp, 1)
                if k >= 2:
                    act.wait_ge(sem_pe, k - 1)   # RB2[h] free (PE k-2 done)
                RBg = RBh.rearrange("p (g q w) -> p g q w", g=ng, q=NCLASS + 1)
                Xg = Xh.rearrange("p (c g w) -> p c g w", c=NCLASS, g=ng)
                act.copy(
                    RBg[:, :, SPLIT_D:NCLASS, :].rearrange("p g q w -> p q g w"),
                    Xg[:, SPLIT_D:NCLASS, :, :],
                ).then_inc(sem_cast, 1)
                act.wait_ge(sem_tree, k + 1)
                # sumexp landed in E[h] block 0 (bf16); lse -> payload q=19 slot
                act.activation(
                    RBg[:, :, NCLASS, :],
                    Eh[:, 0:F].rearrange("p (g w) -> p g w", g=ng),
                    AF.Ln,
                ).then_inc(sem_lse, 1)
            # tail: psum -> sbuf -> dram
            act.wait_ge(sem_pe, NCHUNK)
            act.copy(RES[:], psm[:])
            act.dma_start(out[:, :], RES[:]).then_inc(sem_out, 16)
            act.wait_ge(sem_out, 16)

        @block.vector
        def _(dve):
            dve.memset(RES[:], 0.0)
            for k in range(NCHUNK):
                _, F, ng, Xh, Eh, RBh, Mh, Th, TBh = views(k)
                dve.wait_ge(sem_t, 16 * (k + 1))
                dve.tensor_copy(TBh, Th)
                dve.wait_ge(sem_x, 16 * (k + 1))
                if k >= 2:
                    dve.wait_ge(sem_pe, k - 1)   # RB2[h] and M2[h] free
                RBg = RBh.rearrange("p (g q w) -> p g q w", g=ng, q=NCLASS + 1)
                Xg = Xh.rearrange("p (c g w) -> p c g w", c=NCLASS, g=ng)
                dve.tensor_copy(
                    RBg[:, :, 0:SPLIT_D, :].rearrange("p g q w -> p q g w"),
                    Xg[:, 0:SPLIT_D, :, :],
                ).then_inc(sem_cast, 1)
                Mg = Mh.rearrange("p (g c w) -> p g c w", g=ng, c=NCLASS)
                TBg = TBh.rearrange("p (g w) -> p g w", g=ng)
                for c in range(NCLASS):
                    ins = dve.tensor_scalar(
                        out=Mg[:, :, c, :], in0=TBg[:],
                        scalar1=float(c), scalar2=None, op0=ALU.is_equal,
                        op1=ALU.bypass)
                    if c == NCLASS - 1:
                        ins.then_inc(sem_m, 1)
                # batched bf16 pairwise tree-sum of 19 exp blocks -> block 0
                dve.wait_ge(sem_exp, k + 1)
                Eb = Eh.rearrange("p (c f) -> p c f", c=NCLASS)
                def add(dst, a, b):
                    return dve.tensor_tensor(out=dst, in0=a, in1=b, op=ALU.add)
                # L1: even += odd for 9 pairs (one strided instr, in-place)
                add(Eb[:, 0:18:2, :], Eb[:, 0:18:2, :], Eb[:, 1:18:2, :])
                # L2: {0,4,8,12} += {2,6,10,14}; 16 += 18
                add(Eb[:, 0:16:4, :], Eb[:, 0:16:4, :], Eb[:, 2:16:4, :])
                add(Eb[:, 16, :], Eb[:, 16, :], Eb[:, 18, :])
                # L3: {0,8} += {4,12}
                add(Eb[:, 0:16:8, :], Eb[:, 0:16:8, :], Eb[:, 4:16:8, :])
                # L4: 0 += 8 ; L5: 0 += 16
                add(Eb[:, 0, :], Eb[:, 0, :], Eb[:, 8, :])
                ins = add(Eb[:, 0, :], Eb[:, 0, :], Eb[:, 16, :])
                ins.then_inc(sem_tree, 1)

        @block.tensor
        def _(pe):
            first = True
            for k in range(NCHUNK):
                h, F, ng, _, _, RBh, Mh, _, _ = views(k)
                pe.wait_ge(sem_m, k + 1)
                pe.wait_ge(sem_cast, 2 * (k + 1))
                pe.wait_ge(sem_lse, k + 1)
                for g in range(ng):
                    ins = pe.matmul(
                        psm[:],
                        lhsT=RBh[:, g * SW:(g + 1) * SW],
                        rhs=Mh[:, g * MW:(g + 1) * MW],
                        start=first,
                        stop=(k == NCHUNK - 1 and g == ng - 1),
                    )
                    first = False
                    if g == ng - 1:
                        ins.then_inc(sem_pe, 1)

    return nc


_CACHE = {}


def _get_nc():
    if "nc" not in _CACHE:
        _CACHE["nc"] = _build()
    return _CACHE["nc"]


def _run(logits, target, trace=False):
    nc = _get_nc()
    in_maps = []
    for i in range(NCORES):
        in_maps.append({
            "logits": np.ascontiguousarray(logits[i].reshape(NCLASS, P, COLS)),
            "target": np.ascontiguousarray(target[i].reshape(P, COLS)),
        })
    r = run_bass_kernel_spmd(nc, in_maps, core_ids=list(range(NCORES)), trace=trace)
    return r


def _combine(results, target):
    # psum out layout: rows i = q*W + floc (payload), cols j = c*W + floc'
    # (one-hot); valid entries are the floc == floc' diagonals.
    A = np.zeros(NCLASS, np.float64)
    B = np.zeros(NCLASS, np.float64)
    c19 = np.arange(NCLASS)
    for i in range(NCORES):
        res = results[i]["out"].astype(np.float64)
        for floc in range(W):
            A += res[c19 * W + floc, c19 * W + floc]
            B += res[NCLASS * W + floc, c19 * W + floc]
    N = np.bincount(target.reshape(-1), minlength=NCLASS).astype(np.float64)
    w = np.where(N > 0, (1.0 - BETA) / (1.0 - BETA ** N), 0.0)
    num = float((w * (A - B)).sum())
    den = float((w * N).sum())
    return np.float32(-num / den)


def kernel(logits, target):
    assert logits.shape == (NCORES, NCLASS, 512, 1024) and logits.dtype == np.float32
    assert target.shape == (NCORES, 512, 1024) and target.dtype == np.int32
    r = _run(logits, target, trace=False)
    return _combine(r.results, target)
